# revision 15
# baseline (speedup 1.0000x reference)
"""Local (sliding-window) attention kernel for TRN2, 8 NeuronCores.

Problem: B=32, N=8192, D=64 fp32; WINDOW=128, look_backward=1, look_forward=0,
pad_value=-1.0, softmax over the 256 (prev+own window) keys, no masking.

Sharding: batch rows 32 -> 8 cores x 4 rows (pure data parallel, no comms).

v5 design:
  - Host pre-packs layouts: q/k arrive d-major fp16 with PAIRS of batch rows
    stacked on the 128 partitions (row 2p on partitions 0:64, row 2p+1 on
    64:128). v arrives p-major fp16 with the softmax-denominator ones column
    baked in. Device does zero transposes.
  - sim matmuls for the two rows of a pair use disjoint PE row-groups
    (K=64 contraction at partition offsets 0 and 64) and disjoint PSUM banks,
    so they run concurrently -> sim throughput x2.
  - exp on ScalarE over 3-slot-pair PSUM groups (1536 cols/ACTIVATE).
  - sim and AV emission is interleaved so the PE / ScalarE / VectorE
    pipelines overlap across the whole row instead of phase-by-phase.
  - AV matmuls accumulate [v|1] over the 2 key chunks; DVE reciprocal +
    broadcast-mul normalizes; one whole-row fp16 store per batch row.
"""

import numpy as np
from contextlib import ExitStack

import concourse.bass as bass
import concourse.tile as tile
from concourse import bacc, mybir
from concourse.bass_utils import run_bass_kernel_spmd

F32 = mybir.dt.float32
F16 = mybir.dt.float16
EXP = mybir.ActivationFunctionType.Exp

B, N, D = 32, 8192, 64
W = 128                 # window size (tokens per tile)
NT = N // W             # 64 key/query tiles per batch row
NB = B // 8             # 4 batch rows per core
NP = NB // 2            # 2 row-pairs per core
SCALE = D ** -0.5       # 0.125
PAD = -1.0
GS = 3                  # sim slot-pairs per PSUM group (3 x 512 cols = 3 banks)


def _emit(tc, ctx, qT, kT, v65, o):
    nc = tc.nc

    consts = ctx.enter_context(tc.tile_pool(name="consts", bufs=1))
    # pad lhsT tile: -1.0 (dims x keys) on both partition halves; also the
    # moving operand for the HAM-warmup matmuls
    kT_pad = consts.tile([128, 260], F16, name="kT_pad")
    nc.vector.memset(kT_pad, PAD)
    # [v|1] pad tile for window 0's prev-window values
    v_pad = consts.tile([128, 65], F16, name="v_pad")
    nc.vector.memset(v_pad[:, 0:64], PAD)
    nc.vector.memset(v_pad[:, 64:65], 1.0)

    q_pool = ctx.enter_context(tc.tile_pool(name="qpool", bufs=2))
    k_pool = ctx.enter_context(tc.tile_pool(name="kpool", bufs=2))
    v_pool = ctx.enter_context(tc.tile_pool(name="vpool", bufs=1))
    e_pool = ctx.enter_context(tc.tile_pool(name="epool", bufs=4))
    out_pool = ctx.enter_context(tc.tile_pool(name="outp", bufs=1))
    r_pool = ctx.enter_context(tc.tile_pool(name="rpool", bufs=6))

    sim_ps = ctx.enter_context(tc.tile_pool(name="sim_ps", bufs=2, space="PSUM"))
    av_ps = ctx.enter_context(tc.tile_pool(name="av_ps", bufs=2, space="PSUM"))

    # PE warmup burst overlapping the initial DMA wait (runs back-to-back,
    # no dependencies)
    wp = av_ps.tile([128, 260], F32, name="ag")
    for _ in range(16):
        nc.tensor.matmul(wp, kT_pad[0:64, 0:128], kT_pad[0:64, :],
                         start=True, stop=True)

    ngroups = (NT + 1 + GS - 1) // GS    # 65 slots -> 22 groups of <=3

    # ---- all input loads issued upfront, both pairs.
    # q/k ride the scalar HWDGE ring: its FIFO drains pair 0's small first
    # chunk, then progressively larger ones, then pair 1 — so the SDMA
    # round-robin can't starve the critical first tiles behind bulk loads.
    # v rides the gpsimd ring (needed ~3us after the first sim groups);
    # the sync ring carries only output stores.
    qk_tiles = []
    for p in range(NP):
        qs = q_pool.tile([128, N], F16, name="qs")
        ks = k_pool.tile([128, N], F16, name="ks")
        qk_tiles.append((qs, ks))
    v_tiles = [v_pool.tile([128, NT, 65], F16, name="vs", tag=f"vs{b}")
               for b in range(NB)]
    C1, C2 = 8 * W, 24 * W
    (qs0, ks0), (qs1, ks1) = qk_tiles
    # most-critical first tiles get the gpsimd ring's head (exclusive at t=0)
    nc.gpsimd.dma_start(ks0[:, 0:C1], kT[0][:, 0:C1])
    nc.gpsimd.dma_start(qs0[:, 0:C1], qT[0][:, 0:C1])
    nc.scalar.dma_start(ks0[:, C1:C2], kT[0][:, C1:C2])
    nc.scalar.dma_start(qs0[:, C1:C2], qT[0][:, C1:C2])
    nc.scalar.dma_start(ks0[:, C2:N], kT[0][:, C2:N])
    nc.scalar.dma_start(qs0[:, C2:N], qT[0][:, C2:N])
    nc.scalar.dma_start(ks1, kT[1])
    nc.scalar.dma_start(qs1, qT[1])
    for b in range(NB):
        nc.gpsimd.dma_start(v_tiles[b], v65[b])

    for p in range(NP):
        qs, ks = qk_tiles[p]
        vs = [v_tiles[2 * p], v_tiles[2 * p + 1]]

        # ---- slot s <-> key tile t=s-1; per-lane chunk of a group tile:
        # group g covers slots 3g..3g+2; tile cols =
        #   [lane0: slots | lane1: slots], 256 cols per (lane, slot)
        e_groups = []

        def e_chunk(lane, s, prev):
            g = s // GS
            nsl = min(GS, NT + 1 - GS * g)
            off = 256 * (nsl * lane + (s % GS)) + (128 if prev else 0)
            return e_groups[g][:, off:off + 128]

        og = [out_pool.tile([128, NT, D], F16, name="og", tag=f"og{p}{l}")
              for l in (0, 1)]

        def emit_av(j):
            for lane in (0, 1):
                ag = av_ps.tile([128, 260], F32, name="ag")
                for c in range(4):
                    w = 4 * j + c
                    vprev = v_pad if w == 0 else vs[lane][:, w - 1]
                    nc.tensor.matmul(ag[:, 65 * c:65 * (c + 1)],
                                     e_chunk(lane, w, True), vprev,
                                     start=True, stop=False)
                    nc.tensor.matmul(ag[:, 65 * c:65 * (c + 1)],
                                     e_chunk(lane, w + 1, False), vs[lane][:, w],
                                     start=False, stop=True)
                agv = ag.rearrange("p (w c) -> p w c", c=65)
                r4 = r_pool.tile([128, 4], F32, name="r4")
                nc.vector.reciprocal(r4, agv[:, :, 64])
                nc.vector.tensor_mul(og[lane][:, 4 * j:4 * j + 4],
                                     agv[:, :, 0:64],
                                     r4.unsqueeze(2).broadcast_to((128, 4, 64)))

        done = 0
        stored = [False, False]
        for g in range(ngroups):
            nslots = min(GS, NT + 1 - GS * g)
            sg = sim_ps.tile([128, 512 * nslots], F32, name="sg", tag="simg")
            eg = e_pool.tile([128, 512 * nslots], F16, name="eg", tag="eg")
            e_groups.append(eg)
            if g == 0:
                # slot 0 own-halves are unused; define them for exp
                nc.vector.memset(sg[:, 0:128], 0.0)
                nc.vector.memset(sg[:, 256 * nslots:256 * nslots + 128], 0.0)
            # alternate lanes so adjacent matmuls use disjoint PE row-groups
            # (and disjoint PSUM banks) and overlap in the array
            for c in range(nslots):
                s = GS * g + c
                t = s - 1
                for lane in (0, 1):
                    p0 = 64 * lane
                    loff = 256 * nslots * lane
                    out = sg[:, loff + 256 * c:loff + 256 * (c + 1)]
                    if t < 0:
                        nc.tensor.matmul(out[:, 128:256],
                                         kT_pad[p0:p0 + 64, 0:128],
                                         qs[p0:p0 + 64, 0:W],
                                         start=True, stop=True)
                    else:
                        hi = min(t + 2, NT)
                        nc.tensor.matmul(out[:, 0:W * (hi - t)],
                                         ks[p0:p0 + 64, W * t:W * (t + 1)],
                                         qs[p0:p0 + 64, W * t:W * hi],
                                         start=True, stop=True)
            if GS * g + nslots - 1 < NT:
                nc.scalar.activation(eg, sg, EXP, scale=SCALE)
            else:
                # last group: slot NT's prev-half (window NT) doesn't exist;
                # exp each lane's valid prefix separately
                val = 256 * (nslots - 1) + 128
                for lane in (0, 1):
                    loff = 256 * nslots * lane
                    nc.scalar.activation(eg[:, loff:loff + val],
                                         sg[:, loff:loff + val], EXP,
                                         scale=SCALE)
            # AV for ready window groups: windows of group j need slots
            # <= 4j+4, available once 4j+4 <= 3g+2
            while done < NT // 4 and 4 * done + 4 <= GS * g + nslots - 1:
                emit_av(done)
                done += 1
                if done == NT // 8 and not stored[0]:
                    # first-half stores go out early so the final store is
                    # small and drains quickly at the kernel tail
                    for lane in (0, 1):
                        nc.sync.dma_start(o[2 * p + lane][:, 0:NT // 2],
                                          og[lane][:, 0:NT // 2])
                    stored[0] = True
        while done < NT // 4:
            emit_av(done)
            done += 1

        for lane in (0, 1):
            nc.sync.dma_start(o[2 * p + lane][:, NT // 2:NT],
                              og[lane][:, NT // 2:NT])


_CACHED_NC = None


def _build():
    global _CACHED_NC
    if _CACHED_NC is not None:
        return _CACHED_NC
    nc = bacc.Bacc("TRN2", target_bir_lowering=False, debug=False, num_devices=8)
    qT = nc.dram_tensor("qT", [NP, 128, N], F16, kind="ExternalInput").ap()
    kT = nc.dram_tensor("kT", [NP, 128, N], F16, kind="ExternalInput").ap()
    v65 = nc.dram_tensor("v65", [NB, W, NT, 65], F16, kind="ExternalInput").ap()
    o = nc.dram_tensor("o", [NB, W, NT, D], F16, kind="ExternalOutput").ap()
    with tile.TileContext(nc) as tc, ExitStack() as ctx:
        _emit(tc, ctx, qT, kT, v65, o)
    nc.compile()
    _CACHED_NC = nc
    return nc


def kernel(q, k, v, **run_kwargs):
    # host-side layout prep (not on the device critical path):
    # d-major fp16 q/k with row-pairs stacked on partitions, p-major fp16 v
    # with ones column for the denominator
    qT = np.ascontiguousarray(
        q.astype(np.float16).transpose(0, 2, 1)).reshape(B // 2, 128, N)
    kT = np.ascontiguousarray(
        k.astype(np.float16).transpose(0, 2, 1)).reshape(B // 2, 128, N)
    v16 = v.astype(np.float16).reshape(B, NT, W, D).transpose(0, 2, 1, 3)
    v65 = np.concatenate(
        [v16, np.ones((B, W, NT, 1), dtype=np.float16)], axis=3)
    v65 = np.ascontiguousarray(v65)

    nc = _build()
    in_maps = [
        {"qT": qT[NP * c:NP * (c + 1)], "kT": kT[NP * c:NP * (c + 1)],
         "v65": v65[NB * c:NB * (c + 1)]}
        for c in range(8)
    ]
    res = run_bass_kernel_spmd(nc, in_maps, core_ids=list(range(8)), **run_kwargs)
    out = np.concatenate([res.results[c]["o"] for c in range(8)], axis=0)
    # [B, W, NT, D] p-major fp16 -> [B, N, D] fp32
    out = out.transpose(0, 2, 1, 3).reshape(B, N, D).astype(np.float32)
    if run_kwargs.get("trace"):
        kernel.last_results = res
    return out


# revision 20
# speedup vs baseline: 1.0811x; 1.0811x over previous
"""Local (sliding-window) attention kernel for TRN2, 8 NeuronCores.

Problem: B=32, N=8192, D=64 fp32; WINDOW=128, look_backward=1, look_forward=0,
pad_value=-1.0, softmax over the 256 (prev+own window) keys, no masking.

Sharding: batch rows 32 -> 8 cores x 4 rows (pure data parallel, no comms).

v5 design:
  - Host pre-packs layouts: q/k arrive d-major fp16 with PAIRS of batch rows
    stacked on the 128 partitions (row 2p on partitions 0:64, row 2p+1 on
    64:128). v arrives p-major fp16 with the softmax-denominator ones column
    baked in. Device does zero transposes.
  - sim matmuls for the two rows of a pair use disjoint PE row-groups
    (K=64 contraction at partition offsets 0 and 64) and disjoint PSUM banks,
    so they run concurrently -> sim throughput x2.
  - exp on ScalarE over 3-slot-pair PSUM groups (1536 cols/ACTIVATE).
  - sim and AV emission is interleaved so the PE / ScalarE / VectorE
    pipelines overlap across the whole row instead of phase-by-phase.
  - AV matmuls accumulate [v|1] over the 2 key chunks; DVE reciprocal +
    broadcast-mul normalizes; one whole-row fp16 store per batch row.
"""

import numpy as np
from contextlib import ExitStack

import concourse.bass as bass
import concourse.tile as tile
from concourse import bacc, mybir
from concourse.bass_utils import run_bass_kernel_spmd

F32 = mybir.dt.float32
F16 = mybir.dt.float16
EXP = mybir.ActivationFunctionType.Exp

B, N, D = 32, 8192, 64
W = 128                 # window size (tokens per tile)
NT = N // W             # 64 key/query tiles per batch row
NB = B // 8             # 4 batch rows per core
NP = NB // 2            # 2 row-pairs per core
SCALE = D ** -0.5       # 0.125
PAD = -1.0
GS = 3                  # sim slot-pairs per PSUM group (3 x 512 cols = 3 banks)


def _emit(tc, ctx, qT, kT, v65, o):
    nc = tc.nc

    consts = ctx.enter_context(tc.tile_pool(name="consts", bufs=1))
    # pad lhsT tile: -1.0 (dims x keys) on both partition halves; also the
    # moving operand for the HAM-warmup matmuls
    kT_pad = consts.tile([128, 260], F16, name="kT_pad")
    nc.vector.memset(kT_pad, PAD)
    # [v|1] pad tile for window 0's prev-window values
    v_pad = consts.tile([128, 65], F16, name="v_pad")
    nc.vector.memset(v_pad[:, 0:64], PAD)
    nc.vector.memset(v_pad[:, 64:65], 1.0)

    q_pool = ctx.enter_context(tc.tile_pool(name="qpool", bufs=2))
    k_pool = ctx.enter_context(tc.tile_pool(name="kpool", bufs=2))
    v_pool = ctx.enter_context(tc.tile_pool(name="vpool", bufs=1))
    e_pool = ctx.enter_context(tc.tile_pool(name="epool", bufs=4))
    out_pool = ctx.enter_context(tc.tile_pool(name="outp", bufs=1))
    r_pool = ctx.enter_context(tc.tile_pool(name="rpool", bufs=6))

    sim_ps = ctx.enter_context(tc.tile_pool(name="sim_ps", bufs=2, space="PSUM"))
    av_ps = ctx.enter_context(tc.tile_pool(name="av_ps", bufs=2, space="PSUM"))

    # PE warmup burst overlapping the initial DMA wait (runs back-to-back,
    # no dependencies)
    wp = av_ps.tile([128, 260], F32, name="ag")
    for _ in range(16):
        nc.tensor.matmul(wp, kT_pad[0:64, 0:128], kT_pad[0:64, :],
                         start=True, stop=True)

    ngroups = (NT + 1 + GS - 1) // GS    # 65 slots -> 22 groups of <=3

    # ---- all input loads issued upfront, both pairs.
    # q/k ride the scalar HWDGE ring: its FIFO drains pair 0's small first
    # chunk, then progressively larger ones, then pair 1 — so the SDMA
    # round-robin can't starve the critical first tiles behind bulk loads.
    # v rides the gpsimd ring (needed ~3us after the first sim groups);
    # the sync ring carries only output stores.
    qk_tiles = []
    for p in range(NP):
        qs = q_pool.tile([128, N], F16, name="qs")
        ks = k_pool.tile([128, N], F16, name="ks")
        qk_tiles.append((qs, ks))
    # pair-0 v is split into a small head tile (unblocks the first AV
    # groups early) and a tail tile; pair-1 v is one tile per row
    VC = 16
    v_head = [v_pool.tile([128, VC, 65], F16, name="vh", tag=f"vh{b}")
              for b in range(2)]
    v_tiles = [v_pool.tile([128, NT - VC, 65], F16, name="vs", tag=f"vs{b}")
               if b < 2 else
               v_pool.tile([128, NT, 65], F16, name="vs", tag=f"vs{b}")
               for b in range(NB)]

    def v_win(b, w):  # [v|1] tile slice for window w of batch row b
        if b < 2 and w < VC:
            return v_head[b][:, w]
        return v_tiles[b][:, w - VC if b < 2 else w]
    C1, C2 = 8 * W, 24 * W
    (qs0, ks0), (qs1, ks1) = qk_tiles
    nc.scalar.dma_start(ks0[:, 0:C1], kT[0][:, 0:C1])
    nc.scalar.dma_start(qs0[:, 0:C1], qT[0][:, 0:C1])
    nc.scalar.dma_start(ks0[:, C1:C2], kT[0][:, C1:C2])
    nc.scalar.dma_start(qs0[:, C1:C2], qT[0][:, C1:C2])
    nc.scalar.dma_start(ks0[:, C2:N], kT[0][:, C2:N])
    nc.scalar.dma_start(qs0[:, C2:N], qT[0][:, C2:N])
    nc.scalar.dma_start(ks1, kT[1])
    nc.scalar.dma_start(qs1, qT[1])
    # v ladder: the first windows of pair 0's v unblock the AV pipeline
    nc.gpsimd.dma_start(v_head[0], v65[0][:, 0:VC])
    nc.gpsimd.dma_start(v_head[1], v65[1][:, 0:VC])
    nc.gpsimd.dma_start(v_tiles[0], v65[0][:, VC:NT])
    nc.gpsimd.dma_start(v_tiles[1], v65[1][:, VC:NT])
    nc.gpsimd.dma_start(v_tiles[2], v65[2])
    nc.gpsimd.dma_start(v_tiles[3], v65[3])

    for p in range(NP):
        qs, ks = qk_tiles[p]

        # ---- slot s <-> key tile t=s-1; per-lane chunk of a group tile:
        # group g covers slots 3g..3g+2; tile cols =
        #   [lane0: slots | lane1: slots], 256 cols per (lane, slot)
        e_groups = []

        def e_chunk(lane, s, prev):
            g = s // GS
            nsl = min(GS, NT + 1 - GS * g)
            off = 256 * (nsl * lane + (s % GS)) + (128 if prev else 0)
            return e_groups[g][:, off:off + 128]

        og = [out_pool.tile([128, NT, D], F16, name="og", tag=f"og{p}{l}")
              for l in (0, 1)]

        def emit_av(j):
            for lane in (0, 1):
                ag = av_ps.tile([128, 260], F32, name="ag")
                for c in range(4):
                    w = 4 * j + c
                    vprev = v_pad if w == 0 else v_win(2 * p + lane, w - 1)
                    nc.tensor.matmul(ag[:, 65 * c:65 * (c + 1)],
                                     e_chunk(lane, w, True), vprev,
                                     start=True, stop=False)
                    nc.tensor.matmul(ag[:, 65 * c:65 * (c + 1)],
                                     e_chunk(lane, w + 1, False),
                                     v_win(2 * p + lane, w),
                                     start=False, stop=True)
                agv = ag.rearrange("p (w c) -> p w c", c=65)
                r4 = r_pool.tile([128, 4], F32, name="r4")
                nc.vector.reciprocal(r4, agv[:, :, 64])
                nc.vector.tensor_mul(og[lane][:, 4 * j:4 * j + 4],
                                     agv[:, :, 0:64],
                                     r4.unsqueeze(2).broadcast_to((128, 4, 64)))

        done = 0
        stored = [False, False]
        for g in range(ngroups):
            nslots = min(GS, NT + 1 - GS * g)
            sg = sim_ps.tile([128, 512 * nslots], F32, name="sg", tag="simg")
            eg = e_pool.tile([128, 512 * nslots], F16, name="eg", tag="eg")
            e_groups.append(eg)
            if g == 0:
                # slot 0 own-halves are unused; define them for exp
                nc.vector.memset(sg[:, 0:128], 0.0)
                nc.vector.memset(sg[:, 256 * nslots:256 * nslots + 128], 0.0)
            # alternate lanes so adjacent matmuls use disjoint PE row-groups
            # (and disjoint PSUM banks) and overlap in the array
            for c in range(nslots):
                s = GS * g + c
                t = s - 1
                for lane in (0, 1):
                    p0 = 64 * lane
                    loff = 256 * nslots * lane
                    out = sg[:, loff + 256 * c:loff + 256 * (c + 1)]
                    if t < 0:
                        nc.tensor.matmul(out[:, 128:256],
                                         kT_pad[p0:p0 + 64, 0:128],
                                         qs[p0:p0 + 64, 0:W],
                                         start=True, stop=True)
                    else:
                        hi = min(t + 2, NT)
                        nc.tensor.matmul(out[:, 0:W * (hi - t)],
                                         ks[p0:p0 + 64, W * t:W * (t + 1)],
                                         qs[p0:p0 + 64, W * t:W * hi],
                                         start=True, stop=True)
            if GS * g + nslots - 1 < NT:
                nc.scalar.activation(eg, sg, EXP, scale=SCALE)
            else:
                # last group: slot NT's prev-half (window NT) doesn't exist;
                # exp each lane's valid prefix separately
                val = 256 * (nslots - 1) + 128
                for lane in (0, 1):
                    loff = 256 * nslots * lane
                    nc.scalar.activation(eg[:, loff:loff + val],
                                         sg[:, loff:loff + val], EXP,
                                         scale=SCALE)
            # AV for ready window groups: windows of group j need slots
            # <= 4j+4, available once 4j+4 <= 3g+2
            while done < NT // 4 and 4 * done + 4 <= GS * g + nslots - 1:
                emit_av(done)
                done += 1
                if done == NT // 8 and not stored[0]:
                    # first-half stores go out early so the final store is
                    # small and drains quickly at the kernel tail
                    for lane in (0, 1):
                        nc.sync.dma_start(o[2 * p + lane][:, 0:NT // 2],
                                          og[lane][:, 0:NT // 2])
                    stored[0] = True
        while done < NT // 4:
            emit_av(done)
            done += 1

        for lane in (0, 1):
            nc.sync.dma_start(o[2 * p + lane][:, NT // 2:NT],
                              og[lane][:, NT // 2:NT])


_CACHED_NC = None


def _build():
    global _CACHED_NC
    if _CACHED_NC is not None:
        return _CACHED_NC
    nc = bacc.Bacc("TRN2", target_bir_lowering=False, debug=False, num_devices=8)
    qT = nc.dram_tensor("qT", [NP, 128, N], F16, kind="ExternalInput").ap()
    kT = nc.dram_tensor("kT", [NP, 128, N], F16, kind="ExternalInput").ap()
    v65 = nc.dram_tensor("v65", [NB, W, NT, 65], F16, kind="ExternalInput").ap()
    o = nc.dram_tensor("o", [NB, W, NT, D], F16, kind="ExternalOutput").ap()
    with tile.TileContext(nc) as tc, ExitStack() as ctx:
        _emit(tc, ctx, qT, kT, v65, o)
    nc.compile()
    _CACHED_NC = nc
    return nc


def kernel(q, k, v, **run_kwargs):
    # host-side layout prep (not on the device critical path):
    # d-major fp16 q/k with row-pairs stacked on partitions, p-major fp16 v
    # with ones column for the denominator
    qT = np.ascontiguousarray(
        q.astype(np.float16).transpose(0, 2, 1)).reshape(B // 2, 128, N)
    kT = np.ascontiguousarray(
        k.astype(np.float16).transpose(0, 2, 1)).reshape(B // 2, 128, N)
    v16 = v.astype(np.float16).reshape(B, NT, W, D).transpose(0, 2, 1, 3)
    v65 = np.concatenate(
        [v16, np.ones((B, W, NT, 1), dtype=np.float16)], axis=3)
    v65 = np.ascontiguousarray(v65)

    nc = _build()
    in_maps = [
        {"qT": qT[NP * c:NP * (c + 1)], "kT": kT[NP * c:NP * (c + 1)],
         "v65": v65[NB * c:NB * (c + 1)]}
        for c in range(8)
    ]
    res = run_bass_kernel_spmd(nc, in_maps, core_ids=list(range(8)), **run_kwargs)
    out = np.concatenate([res.results[c]["o"] for c in range(8)], axis=0)
    # [B, W, NT, D] p-major fp16 -> [B, N, D] fp32
    out = out.transpose(0, 2, 1, 3).reshape(B, N, D).astype(np.float32)
    if run_kwargs.get("trace"):
        kernel.last_results = res
    return out


# revision 24
# speedup vs baseline: 1.1086x; 1.0255x over previous
"""Local (sliding-window) attention kernel for TRN2, 8 NeuronCores.

Problem: B=32, N=8192, D=64 fp32; WINDOW=128, look_backward=1, look_forward=0,
pad_value=-1.0, softmax over the 256 (prev+own window) keys, no masking.

Sharding: batch rows 32 -> 8 cores x 4 rows (pure data parallel, no comms).

v5 design:
  - Host pre-packs layouts: q/k arrive d-major fp16 with PAIRS of batch rows
    stacked on the 128 partitions (row 2p on partitions 0:64, row 2p+1 on
    64:128). v arrives p-major fp16 with the softmax-denominator ones column
    baked in. Device does zero transposes.
  - sim matmuls for the two rows of a pair use disjoint PE row-groups
    (K=64 contraction at partition offsets 0 and 64) and disjoint PSUM banks,
    so they run concurrently -> sim throughput x2.
  - exp on ScalarE over 3-slot-pair PSUM groups (1536 cols/ACTIVATE).
  - sim and AV emission is interleaved so the PE / ScalarE / VectorE
    pipelines overlap across the whole row instead of phase-by-phase.
  - AV matmuls accumulate [v|1] over the 2 key chunks; DVE reciprocal +
    broadcast-mul normalizes; one whole-row fp16 store per batch row.
"""

import numpy as np
from contextlib import ExitStack

import concourse.bass as bass
import concourse.tile as tile
from concourse import bacc, mybir
from concourse.bass_utils import run_bass_kernel_spmd

F32 = mybir.dt.float32
F16 = mybir.dt.float16
EXP = mybir.ActivationFunctionType.Exp

B, N, D = 32, 8192, 64
W = 128                 # window size (tokens per tile)
NT = N // W             # 64 key/query tiles per batch row
NB = B // 8             # 4 batch rows per core
NP = NB // 2            # 2 row-pairs per core
SCALE = D ** -0.5       # 0.125
PAD = -1.0
GS = 3                  # sim slot-pairs per PSUM group (3 x 512 cols = 3 banks)


def _emit(tc, ctx, qT, kT, v65, o):
    nc = tc.nc

    consts = ctx.enter_context(tc.tile_pool(name="consts", bufs=1))
    # pad lhsT tile: -1.0 (dims x keys) on both partition halves; also the
    # moving operand for the HAM-warmup matmuls
    kT_pad = consts.tile([128, 260], F16, name="kT_pad")
    nc.vector.memset(kT_pad, PAD)
    # [v|1] pad tile for window 0's prev-window values
    v_pad = consts.tile([128, 65], F16, name="v_pad")
    nc.vector.memset(v_pad[:, 0:64], PAD)
    nc.vector.memset(v_pad[:, 64:65], 1.0)

    q_pool = ctx.enter_context(tc.tile_pool(name="qpool", bufs=2))
    k_pool = ctx.enter_context(tc.tile_pool(name="kpool", bufs=2))
    v_pool = ctx.enter_context(tc.tile_pool(name="vpool", bufs=1))
    e_pool = ctx.enter_context(tc.tile_pool(name="epool", bufs=4))
    out_pool = ctx.enter_context(tc.tile_pool(name="outp", bufs=1))
    r_pool = ctx.enter_context(tc.tile_pool(name="rpool", bufs=6))

    sim_ps = ctx.enter_context(tc.tile_pool(name="sim_ps", bufs=2, space="PSUM"))
    av_ps = ctx.enter_context(tc.tile_pool(name="av_ps", bufs=2, space="PSUM"))

    # PE warmup burst overlapping the initial DMA wait (runs back-to-back,
    # no dependencies)
    wp = av_ps.tile([128, 260], F32, name="ag")
    for _ in range(16):
        nc.tensor.matmul(wp, kT_pad[0:64, 0:128], kT_pad[0:64, :],
                         start=True, stop=True)

    ngroups = (NT + 1 + GS - 1) // GS    # 65 slots -> 22 groups of <=3

    # ---- all input loads issued upfront, both pairs.
    # q/k ride the scalar HWDGE ring: its FIFO drains pair 0's small first
    # chunk, then progressively larger ones, then pair 1 — so the SDMA
    # round-robin can't starve the critical first tiles behind bulk loads.
    # v rides the gpsimd ring (needed ~3us after the first sim groups);
    # the sync ring carries only output stores.
    qk_tiles = []
    for p in range(NP):
        qs = q_pool.tile([128, N], F16, name="qs")
        ks = k_pool.tile([128, N], F16, name="ks")
        qk_tiles.append((qs, ks))
    # pair-0 v is split into head/mid/tail tiles so early AV groups unblock
    # as soon as possible; pair-1 v is one tile per row
    v_chunks = {b: [(0, 16), (16, 24), (40, 24)] if b < 2 else [(0, NT)]
                for b in range(NB)}
    v_tiles = {b: [v_pool.tile([128, ln, 65], F16, name="vs", tag=f"vs{b}_{st}")
                   for st, ln in v_chunks[b]]
               for b in range(NB)}

    def v_win(b, w):  # [v|1] tile slice for window w of batch row b
        for (st, ln), tl in zip(v_chunks[b], v_tiles[b]):
            if st <= w < st + ln:
                return tl[:, w - st]
        raise AssertionError
    # scalar HWDGE ring in deadline order: pair-0 q/k ladder, then pair-1's
    # v rows, then pair-1 q/k. gpsimd ring: only pair-0's v ladder.
    C1, C2 = 8 * W, 24 * W
    (qs0, ks0), (qs1, ks1) = qk_tiles
    nc.scalar.dma_start(ks0[:, 0:C1], kT[0][:, 0:C1])
    nc.scalar.dma_start(qs0[:, 0:C1], qT[0][:, 0:C1])
    for b, (st, ln) in [(0, v_chunks[0][0]), (1, v_chunks[1][0])]:
        nc.gpsimd.dma_start(v_tiles[b][0], v65[b][:, st:st + ln])
    nc.scalar.dma_start(ks0[:, C1:C2], kT[0][:, C1:C2])
    nc.scalar.dma_start(qs0[:, C1:C2], qT[0][:, C1:C2])
    for ci in (1, 2):
        for b in (0, 1):
            st, ln = v_chunks[b][ci]
            nc.gpsimd.dma_start(v_tiles[b][ci], v65[b][:, st:st + ln])
    nc.scalar.dma_start(ks0[:, C2:N], kT[0][:, C2:N])
    nc.scalar.dma_start(qs0[:, C2:N], qT[0][:, C2:N])
    nc.scalar.dma_start(v_tiles[2][0], v65[2])
    nc.scalar.dma_start(v_tiles[3][0], v65[3])
    nc.scalar.dma_start(ks1, kT[1])
    nc.scalar.dma_start(qs1, qT[1])

    for p in range(NP):
        qs, ks = qk_tiles[p]

        # ---- slot s <-> key tile t=s-1; per-lane chunk of a group tile:
        # group g covers slots 3g..3g+2; tile cols =
        #   [lane0: slots | lane1: slots], 256 cols per (lane, slot)
        e_groups = []

        def e_chunk(lane, s, prev):
            g = s // GS
            nsl = min(GS, NT + 1 - GS * g)
            off = 256 * (nsl * lane + (s % GS)) + (128 if prev else 0)
            return e_groups[g][:, off:off + 128]

        og = [out_pool.tile([128, NT, D], F16, name="og", tag=f"og{p}{l}")
              for l in (0, 1)]

        def emit_av(j):
            for lane in (0, 1):
                ag = av_ps.tile([128, 260], F32, name="ag")
                for c in range(4):
                    w = 4 * j + c
                    vprev = v_pad if w == 0 else v_win(2 * p + lane, w - 1)
                    nc.tensor.matmul(ag[:, 65 * c:65 * (c + 1)],
                                     e_chunk(lane, w, True), vprev,
                                     start=True, stop=False)
                    nc.tensor.matmul(ag[:, 65 * c:65 * (c + 1)],
                                     e_chunk(lane, w + 1, False),
                                     v_win(2 * p + lane, w),
                                     start=False, stop=True)
                agv = ag.rearrange("p (w c) -> p w c", c=65)
                r4 = r_pool.tile([128, 4], F32, name="r4")
                nc.vector.reciprocal(r4, agv[:, :, 64])
                nc.vector.tensor_mul(og[lane][:, 4 * j:4 * j + 4],
                                     agv[:, :, 0:64],
                                     r4.unsqueeze(2).broadcast_to((128, 4, 64)))

        def emit_store(qr):
            # store finished 16-window quarters; lanes go on different rings
            # so the final (small) stores drain in parallel at the tail
            lo, hi = 16 * qr, 16 * qr + 16
            nc.sync.dma_start(o[2 * p][:, lo:hi], og[0][:, lo:hi])
            nc.gpsimd.dma_start(o[2 * p + 1][:, lo:hi], og[1][:, lo:hi])

        done = 0
        for g in range(ngroups):
            nslots = min(GS, NT + 1 - GS * g)
            sg = sim_ps.tile([128, 512 * nslots], F32, name="sg", tag="simg")
            eg = e_pool.tile([128, 512 * nslots], F16, name="eg", tag="eg")
            e_groups.append(eg)
            if g == 0:
                # slot 0 own-halves are unused; define them for exp
                nc.vector.memset(sg[:, 0:128], 0.0)
                nc.vector.memset(sg[:, 256 * nslots:256 * nslots + 128], 0.0)
            # alternate lanes so adjacent matmuls use disjoint PE row-groups
            # (and disjoint PSUM banks) and overlap in the array
            for c in range(nslots):
                s = GS * g + c
                t = s - 1
                for lane in (0, 1):
                    p0 = 64 * lane
                    loff = 256 * nslots * lane
                    out = sg[:, loff + 256 * c:loff + 256 * (c + 1)]
                    if t < 0:
                        nc.tensor.matmul(out[:, 128:256],
                                         kT_pad[p0:p0 + 64, 0:128],
                                         qs[p0:p0 + 64, 0:W],
                                         start=True, stop=True)
                    else:
                        hi = min(t + 2, NT)
                        nc.tensor.matmul(out[:, 0:W * (hi - t)],
                                         ks[p0:p0 + 64, W * t:W * (t + 1)],
                                         qs[p0:p0 + 64, W * t:W * hi],
                                         start=True, stop=True)
            if GS * g + nslots - 1 < NT:
                nc.scalar.activation(eg, sg, EXP, scale=SCALE)
            else:
                # last group: slot NT's prev-half (window NT) doesn't exist;
                # exp each lane's valid prefix separately
                val = 256 * (nslots - 1) + 128
                for lane in (0, 1):
                    loff = 256 * nslots * lane
                    nc.scalar.activation(eg[:, loff:loff + val],
                                         sg[:, loff:loff + val], EXP,
                                         scale=SCALE)
            # AV for ready window groups: windows of group j need slots
            # <= 4j+4, available once 4j+4 <= 3g+2
            while done < NT // 4 and 4 * done + 4 <= GS * g + nslots - 1:
                emit_av(done)
                done += 1
                if done % 4 == 0:
                    emit_store(done // 4 - 1)
        while done < NT // 4:
            emit_av(done)
            done += 1
            if done % 4 == 0:
                emit_store(done // 4 - 1)


_CACHED_NC = None


def _build():
    global _CACHED_NC
    if _CACHED_NC is not None:
        return _CACHED_NC
    nc = bacc.Bacc("TRN2", target_bir_lowering=False, debug=False, num_devices=8)
    qT = nc.dram_tensor("qT", [NP, 128, N], F16, kind="ExternalInput").ap()
    kT = nc.dram_tensor("kT", [NP, 128, N], F16, kind="ExternalInput").ap()
    v65 = nc.dram_tensor("v65", [NB, W, NT, 65], F16, kind="ExternalInput").ap()
    o = nc.dram_tensor("o", [NB, W, NT, D], F16, kind="ExternalOutput").ap()
    with tile.TileContext(nc) as tc, ExitStack() as ctx:
        _emit(tc, ctx, qT, kT, v65, o)
    nc.compile()
    _CACHED_NC = nc
    return nc


def kernel(q, k, v, **run_kwargs):
    # host-side layout prep (not on the device critical path):
    # d-major fp16 q/k with row-pairs stacked on partitions, p-major fp16 v
    # with ones column for the denominator
    qT = np.ascontiguousarray(
        q.astype(np.float16).transpose(0, 2, 1)).reshape(B // 2, 128, N)
    kT = np.ascontiguousarray(
        k.astype(np.float16).transpose(0, 2, 1)).reshape(B // 2, 128, N)
    v16 = v.astype(np.float16).reshape(B, NT, W, D).transpose(0, 2, 1, 3)
    v65 = np.concatenate(
        [v16, np.ones((B, W, NT, 1), dtype=np.float16)], axis=3)
    v65 = np.ascontiguousarray(v65)

    nc = _build()
    in_maps = [
        {"qT": qT[NP * c:NP * (c + 1)], "kT": kT[NP * c:NP * (c + 1)],
         "v65": v65[NB * c:NB * (c + 1)]}
        for c in range(8)
    ]
    res = run_bass_kernel_spmd(nc, in_maps, core_ids=list(range(8)), **run_kwargs)
    out = np.concatenate([res.results[c]["o"] for c in range(8)], axis=0)
    # [B, W, NT, D] p-major fp16 -> [B, N, D] fp32
    out = out.transpose(0, 2, 1, 3).reshape(B, N, D).astype(np.float32)
    if run_kwargs.get("trace"):
        kernel.last_results = res
    return out
